# revision 30
# baseline (speedup 1.0000x reference)
"""NonLocal2D block (SAGAN-style non-local attention) on 8 Trainium2 cores.

Data-parallel over batch: core b computes batch element b entirely on-chip.

Math (per batch, N = 64*64 = 4096):
  f = Wf@x+bf [16,N], g = Wg@x+bg [16,N], h = Wh@x+bh [128,N]
  S = f^T g [N,N]; A = softmax_rows(S); att = h @ A; out = x + gamma*att

Design notes (engine/PSUM constraints: GPSIMD cannot touch PSUM; PE is
in-order; PSUM = 8 banks):
  * PE: S and f/g/h projections via fp8e4 DoubleRow matmuls (0.5
    cyc/col, exact in PSUM fp32); attended stays bf16.
  * exp split: 20 "A" strips on ACT (true exp, 1024-chunks from a
    4-bank pool) and 12 "D" strips on DVE via the Schraudolph bf16
    bit-trick (i16 = S*2^7/ln2 + b, bits = bf16 exp; ~3% rel err,
    well inside tolerance; softmax normalization uses the same values
    so the bias largely cancels), 512-chunks from a 2-bank pool.  The
    two streams self-pace on separate PSUM rotations; emission is
    paced by estimated time so PE (in-order) never queues far ahead.
  * D row-sums: ACT accum (8 strips), gpsimd in-place accum pass over
    E in SBUF (14 strips), DVE in-place accum (10 strips).
  * attended: PSUM K-chains over strip groups (8,8,6,6,4), emitted as
    2-matmul segments ~4 per strip one group later; 40 folds on DVE;
    residual added in the first fold (bf16 x); hts scaling on gpsimd.
"""

import numpy as np
import ml_dtypes

import concourse.bass as bass
import concourse.bacc as bacc
import concourse.tile as tile
import concourse.mybir as mybir
from concourse.bass_utils import run_bass_kernel_spmd

B, C, W, H = 8, 128, 64, 64
N = W * H          # 4096
P = 128
NSTRIP = 32
MBLK = 512
NMB = 8

F32 = mybir.dt.float32
BF16 = mybir.dt.bfloat16
I16 = mybir.dt.int16
FP8 = mybir.dt.float8e4
EXP = mybir.ActivationFunctionType.Exp
MUL = mybir.AluOpType.mult
ADD = mybir.AluOpType.add
DR = mybir.MatmulPerfMode.DoubleRow

# Schraudolph exp -> bf16 bits: i16 = trunc(x*SA + SB); SB folds the
# softmax shift (-6, matching the ACT strips) and the error-centering
# delta.
SA = 128.0 / np.log(2.0)
SB = 127.0 * 128.0 - 6.0 * SA - 5.51

# exp engine per strip: A=ACT, D=DVE
_WPAT = {0: "ADAD", 1: "ADAA", 2: "ADAD", 3: "ADAA",
         4: "ADAD", 5: "ADAA", 6: "ADAD", 7: "ADAA"}
COLORS = [c for w in range(8) for c in _WPAT[w]]

# strip groups for the attended K-chains and their consumer spans
GROUPS = [list(range(0, 8)), list(range(8, 16)), list(range(16, 22)),
          list(range(22, 28)), list(range(28, 32))]
SPANS = [list(range(8, 16)), list(range(16, 22)), list(range(22, 28)),
         list(range(28, 32)), None]  # None -> tail


def _gi_of(s):
    for gi, g in enumerate(GROUPS):
        if s in g:
            return gi
    raise AssertionError


def _d_loc(s):
    """Where strip s's row-sum D is computed."""
    if COLORS[s] == "A":
        return "acc" if s % 4 == 0 else "pool"
    return "pool" if s in (1, 9, 17, 25) else "dve"


_NC = None


def _build():
    nc = bacc.Bacc(None, target_bir_lowering=False)
    # x8: fp8 x in DR layout, per block j cols j*1024 + ks*512 + i
    x8 = nc.dram_tensor("x8", [64, 2 * N], FP8, kind="ExternalInput")
    xb = nc.dram_tensor("xb", [P, N], BF16, kind="ExternalInput")
    # w8: [64, 2, 160]: per ks: wfT_lo[0:8] wfT_hi[8:16] wgT_lo[16:24]
    #     wgT_hi[24:32] whT[32:160]
    w8 = nc.dram_tensor("w8", [64, 2 * 160], FP8, kind="ExternalInput")
    bh16 = nc.dram_tensor("bh16", [1, P], BF16, kind="ExternalInput")
    # fpack: col0 bf_lo col1 bf_hi col2 bg_lo col3 bg_hi (rows 0-7), col4 gamma
    fpack = nc.dram_tensor("fpack", [P, 5], F32, kind="ExternalInput")
    out = nc.dram_tensor("out", [P, N], F32, kind="ExternalOutput")

    with tile.TileContext(nc) as tc:
        with (
            tc.tile_pool(name="consts", bufs=1) as consts,
            tc.tile_pool(name="epool", bufs=17) as epool,
            tc.tile_pool(name="hpool", bufs=16) as hpool,
            tc.tile_pool(name="hraw", bufs=4) as hraw,
            tc.tile_pool(name="small", bufs=12) as small,
            tc.tile_pool(name="tree", bufs=2) as tree,
            tc.tile_pool(name="psS", bufs=2, space="PSUM") as psS,
            tc.tile_pool(name="psX", bufs=2, space="PSUM") as psX,
            tc.tile_pool(name="psF", bufs=2, space="PSUM") as psF,
        ):
            # ---- constants + input DMAs -------------------------------
            w8_s = consts.tile([64, 2, 160], FP8)
            nc.sync.dma_start(w8_s[:, 0, :], w8[:, 0:160])
            nc.sync.dma_start(w8_s[:, 1, :], w8[:, 160:320])
            fpack_s = consts.tile([P, 5], F32)
            nc.gpsimd.dma_start(fpack_s[:], fpack[:])
            bh_s = consts.tile([1, P], BF16)
            nc.gpsimd.dma_start(bh_s[:], bh16[:])

            x8_t = [consts.tile([64, 2, MBLK], FP8, tag=f"x8{j}",
                                name=f"x8{j}") for j in range(NMB)]
            _x8eng = [nc.sync, nc.gpsimd, nc.sync, nc.gpsimd,
                      nc.scalar, nc.scalar, nc.sync, nc.gpsimd]
            for j in range(NMB):
                _x8eng[j].dma_start(x8_t[j][:],
                                    x8[:, j * 1024:(j + 1) * 1024])

            wf_lo = w8_s[:, :, 0:8]
            wf_hi = w8_s[:, :, 8:16]
            wg_lo = w8_s[:, :, 16:24]
            wg_hi = w8_s[:, :, 24:32]
            wht = w8_s[:, :, 32:160]
            bf_lo = fpack_s[0:8, 0:1]
            bf_hi = fpack_s[0:8, 1:2]
            bg_lo = fpack_s[0:8, 2:3]
            bg_hi = fpack_s[0:8, 3:4]
            gam_s = fpack_s[:, 4:5]

            ones_s = consts.tile([1, P], BF16)
            nc.vector.memset(ones_s[:], 1.0)
            neg6_s = consts.tile([P, 1], F32)
            nc.vector.memset(neg6_s[:], -6.0)
            # dummy exp pulls the ACT table load to t=0
            warm = small.tile([P, 1], F32, tag="warm")
            nc.scalar.activation(out=warm[:], in_=neg6_s[:], func=EXP)

            fdr_t = [consts.tile([8, 2, MBLK], FP8, tag=f"fdr{b}",
                                 name=f"fdr{b}") for b in range(NMB)]
            gdr_t = [consts.tile([8, 2, 1024], FP8, tag=f"gdr{c}",
                                 name=f"gdr{c}") for c in range(4)]
            att_t = [consts.tile([P, MBLK], F32, tag=f"att{j}",
                                 name=f"att{j}") for j in range(NMB)]

            _cp = {"i": 0}

            def copy_eng():
                _cp["i"] += 1
                return nc.vector if _cp["i"] % 2 == 0 else nc.scalar

            def fg_copy(eng, dst, src, bias):
                if eng is nc.scalar:
                    nc.scalar.activation(
                        out=dst, in_=src,
                        func=mybir.ActivationFunctionType.Identity,
                        bias=bias, scale=1.0)
                else:
                    nc.vector.tensor_scalar_add(out=dst, in0=src,
                                                scalar1=bias)

            def fg_proj(kind, b):
                """f or g projection for 512-block b, fp8 DR, DR layout out."""
                ps = psS.tile([P, 1024], F32, tag="sA", name=f"fg{kind}{b}")
                lo, hi = (wf_lo, wf_hi) if kind == "f" else (wg_lo, wg_hi)
                blo, bhi = (bf_lo, bf_hi) if kind == "f" else (bg_lo, bg_hi)
                nc.tensor.matmul(ps[0:8, 0:MBLK], lo, x8_t[b][:],
                                 start=True, stop=True, perf_mode=DR)
                nc.tensor.matmul(ps[0:8, MBLK:2 * MBLK], hi, x8_t[b][:],
                                 start=True, stop=True, perf_mode=DR)
                if kind == "f":
                    d0 = fdr_t[b][:, 0, :]
                    d1 = fdr_t[b][:, 1, :]
                else:
                    ci, o = b // 2, (b % 2) * MBLK
                    d0 = gdr_t[ci][:, 0, o:o + MBLK]
                    d1 = gdr_t[ci][:, 1, o:o + MBLK]
                fg_copy(copy_eng(), d0, ps[0:8, 0:MBLK], blo)
                fg_copy(copy_eng(), d1, ps[0:8, MBLK:2 * MBLK], bhi)

            # xb (bf16 residual), per block
            xb_t = []
            for j in range(NMB):
                t = consts.tile([P, MBLK], BF16, tag=f"xb{j}", name=f"xb{j}")
                nc.sync.dma_start(t[:], xb[:, j * MBLK:(j + 1) * MBLK])
                xb_t.append(t)

            # ---- attended chains --------------------------------------
            groups = [[] for _ in range(len(GROUPS))]
            _open = {}
            _deferred = []

            def att_seg(j, gi, ks, first, last):
                if len(groups[gi]) < ks[-1] + 1:
                    _deferred.append((j, gi, ks, first, last))
                    return
                grp = groups[gi]
                blk = slice(j * MBLK, (j + 1) * MBLK)
                if ks[0] == 0:
                    _open[j] = psF.tile([P, MBLK], F32, tag="att",
                                        name=f"pa{j}")
                pa = _open[j]
                for k in ks:
                    hk, ek = grp[k]
                    nc.tensor.matmul(pa[:], hk[:], ek[:, blk],
                                     start=(k == 0),
                                     stop=(k == len(grp) - 1))
                if ks[-1] == len(grp) - 1:
                    nc.vector.tensor_tensor(
                        out=att_t[j][:], in0=pa[:],
                        in1=(xb_t[j][:] if first else att_t[j][:]), op=ADD)
                    if last:
                        nc.sync.dma_start(out[:, blk], att_t[j][:])

            # per-strip segment schedule: group gi's chains spread over
            # its span, ~(#segs/#span-strips) segs per strip
            seg_sched = {s: [] for s in range(NSTRIP)}
            for gi, span in enumerate(SPANS):
                if span is None:
                    continue
                glen = len(GROUPS[gi])
                nseg_c = glen // 2          # segments per chain
                segs = []
                for j in range(NMB):
                    for t in range(nseg_c):
                        segs.append((j, gi, (2 * t, 2 * t + 1), gi == 0,
                                     False))
                per = len(segs) / len(span)
                for i, s in enumerate(span):
                    lo = int(round(i * per))
                    hi = int(round((i + 1) * per))
                    seg_sched[s] = segs[lo:hi]

            # ---- per-strip emission helpers ---------------------------
            def a_chunk(s, e, cix, accs):
                i = s % 4
                fsl = fdr_t[s // 4][:, :, i * P:(i + 1) * P]
                sps = psS.tile([P, 1024], F32, tag="sA", name="spsA")
                for half in range(2):
                    off = half * MBLK
                    nc.tensor.matmul(
                        sps[:, off:off + MBLK], fsl,
                        gdr_t[cix][:, :, off:off + MBLK],
                        start=True, stop=True, perf_mode=DR)
                eout = e[:, cix * 1024:(cix + 1) * 1024]
                if accs is None:
                    nc.scalar.activation(out=eout, in_=sps[:], func=EXP,
                                         bias=neg6_s[:])
                else:
                    nc.scalar.activation(out=eout, in_=sps[:], func=EXP,
                                         bias=neg6_s[:],
                                         accum_out=accs[:, cix:cix + 1])

            def x_chunk(s, e, k):
                i = s % 4
                fsl = fdr_t[s // 4][:, :, i * P:(i + 1) * P]
                sps = psX.tile([P, MBLK], F32, tag="sX", name="spsX")
                nc.tensor.matmul(
                    sps[:], fsl,
                    gdr_t[k // 2][:, :, (k % 2) * MBLK:(k % 2 + 1) * MBLK],
                    start=True, stop=True, perf_mode=DR)
                nc.vector.tensor_scalar(
                    out=e[:, k * MBLK:(k + 1) * MBLK].bitcast(I16),
                    in0=sps[:], scalar1=SA, scalar2=SB, op0=MUL, op1=ADD)

            _ht0 = {}

            def strip_head(s):
                """h projection -> ht0 for strip s."""
                i = s % 4
                ph = psX.tile([P, MBLK], F32, tag="sX", name="ph")
                nc.tensor.matmul(ph[:, 0:P],
                                 x8_t[s // 4][:, :, i * P:(i + 1) * P],
                                 wht, start=True, stop=False, perf_mode=DR)
                nc.tensor.matmul(ph[:, 0:P], ones_s[:], bh_s[:],
                                 start=False, stop=True)
                ht0 = hraw.tile([P, P], BF16, tag="ht0", name="ht0")
                nc.vector.tensor_copy(out=ht0[:], in_=ph[:, 0:P])
                _ht0[s] = ht0

            def strip_tail(s, e, accs):
                """D row-sum, hts for strip s."""
                ht0 = _ht0.pop(s)
                d = small.tile([P, 1], F32, tag="d")
                loc = _d_loc(s)
                if loc == "acc":
                    nc.vector.scalar_tensor_tensor(
                        out=d[:], in0=accs[:, 0:1], scalar=accs[:, 1:2],
                        op0=ADD, in1=accs[:, 2:3], op1=ADD)
                    nc.vector.scalar_tensor_tensor(
                        out=d[:], in0=d[:], scalar=0.0,
                        op0=ADD, in1=accs[:, 3:4], op1=ADD)
                elif loc == "pool":
                    # Pool does the halving add (SBUF only); DVE finishes
                    # with a 2048-wide in-place accum pass at 4x
                    t1 = tree.tile([P, 2048], BF16, tag="t1")
                    nc.gpsimd.tensor_add(out=t1[:], in0=e[:, 0:2048],
                                         in1=e[:, 2048:4096])
                    nc.vector.tensor_scalar(out=t1[:], in0=t1[:], scalar1=1.0,
                                            scalar2=0.0, op0=MUL, op1=ADD,
                                            accum_out=d[:])
                else:
                    nc.vector.tensor_scalar(out=e[:], in0=e[:], scalar1=1.0,
                                            scalar2=0.0, op0=MUL, op1=ADD,
                                            accum_out=d[:])
                rd = small.tile([P, 1], F32, tag="rd")
                nc.vector.reciprocal_approx_fast(out=rd[:], in_=d[:])
                hts = hpool.tile([P, P], BF16, tag="hts")
                nc.gpsimd.tensor_scalar(out=hts[:], in0=ht0[:],
                                        scalar1=rd[:], scalar2=gam_s,
                                        op0=MUL, op1=MUL)
                groups[_gi_of(s)].append((hts, e))
                if _deferred:
                    pend, _deferred[:] = _deferred[:], []
                    for it in pend:
                        att_seg(*it)

            # ---- main loop: per-window estimated-time interleave ------
            e_t, acc_t = {}, {}
            gdr_done = set()

            def need_gdr(ci):
                if ci not in gdr_done:
                    gdr_done.add(ci)
                    fg_proj("g", 2 * ci)
                    fg_proj("g", 2 * ci + 1)

            fg_proj("f", 0)
            gui = [0]
            tails_due = []
            for w in range(8):
                strips = [4 * w + i for i in range(4)]
                for s in strips:
                    e_t[s] = epool.tile([P, N], BF16, tag="E", name=f"e{s}")
                    acc_t[s] = (small.tile([P, 4], F32, tag="accs",
                                           name=f"ac{s}")
                                if _d_loc(s) == "acc" else None)
                units = []
                n_a = 0
                for s in strips:
                    if COLORS[s] == "A":
                        base = n_a * 4 * 1.25
                        n_a += 1
                        for c in range(4):
                            units.append((base + c * 1.25, s, "a", c))
                    else:
                        for k in range(8):
                            units.append((1.0 + k * 0.80, s, "x", k))
                units.sort(key=lambda u: (u[0], u[1]))
                # this window's chain segments, paced across units
                segs = [sg for s in strips for sg in seg_sched[s]]
                nseg, nunit = len(segs), len(units)
                emitted = {s: 0 for s in strips}
                segi = 0
                for ui, (pos, s, kind, c) in enumerate(units):
                    while tails_due and tails_due[0][0] <= gui[0]:
                        _, s2 = tails_due.pop(0)
                        strip_tail(s2, e_t[s2], acc_t[s2])
                    if kind == "a":
                        need_gdr(c)
                        a_chunk(s, e_t[s], c, acc_t[s])
                    else:
                        need_gdr(c // 2)
                        x_chunk(s, e_t[s], c)
                    gui[0] += 1
                    while segi < nseg and segi <= (ui + 1) * nseg / nunit - 0.01:
                        att_seg(*segs[segi])
                        segi += 1
                    emitted[s] += 1
                    if emitted[s] == (4 if COLORS[s] == "A" else 8):
                        strip_head(s)
                        tails_due.append((gui[0] + 6, s))
                    if ui == 5 and w < 7:
                        fg_proj("f", w + 1)
                while segi < nseg:
                    att_seg(*segs[segi])
                    segi += 1
            for _, s2 in tails_due:
                strip_tail(s2, e_t[s2], acc_t[s2])

            # ---- tail: last group's chains, folds + stores ------------
            grp = groups[len(GROUPS) - 1]
            for j in range(NMB):
                pool, tg = (psS, "sA") if j % 2 == 0 else (psF, "att")
                pa = pool.tile([P, 1024 if j % 2 == 0 else MBLK], F32,
                               tag=tg, name=f"tailpa{j}")
                for k, (hk, ek) in enumerate(grp):
                    nc.tensor.matmul(pa[:, 0:MBLK], hk[:],
                                     ek[:, j * MBLK:(j + 1) * MBLK],
                                     start=(k == 0), stop=(k == len(grp) - 1))
                nc.vector.tensor_tensor(out=att_t[j][:], in0=att_t[j][:],
                                        in1=pa[:, 0:MBLK], op=ADD)
                nc.sync.dma_start(out[:, j * MBLK:(j + 1) * MBLK], att_t[j][:])

    nc.compile()
    return nc


def _get_nc():
    global _NC
    if _NC is None:
        _NC = _build()
    return _NC


def _prep_weights(Wf, bf, Wg, bg, Wh, bh, gamma):
    fp8 = ml_dtypes.float8_e4m3
    bf16 = ml_dtypes.bfloat16
    w8 = np.zeros((64, 2, 160), np.float32)
    wfT, wgT, whT = Wf.T, Wg.T, Wh.T
    for ks in range(2):
        rows = slice(ks * 64, (ks + 1) * 64)
        w8[:, ks, 0:8] = wfT[rows, 0:8]
        w8[:, ks, 8:16] = wfT[rows, 8:16]
        w8[:, ks, 16:24] = wgT[rows, 0:8]
        w8[:, ks, 24:32] = wgT[rows, 8:16]
        w8[:, ks, 32:160] = whT[rows, :]
    fpack = np.zeros((P, 5), np.float32)
    fpack[0:8, 0] = bf[0:8]
    fpack[0:8, 1] = bf[8:16]
    fpack[0:8, 2] = bg[0:8]
    fpack[0:8, 3] = bg[8:16]
    fpack[:, 4] = np.float32(np.asarray(gamma).reshape(()))
    return {
        "w8": w8.reshape(64, 320).astype(fp8),
        "bh16": bh.reshape(1, P).astype(bf16),
        "fpack": fpack,
    }


def make_in_maps(x, Wf, bf, Wg, bg, Wh, bh, gamma):
    fp8 = ml_dtypes.float8_e4m3
    bf16 = ml_dtypes.bfloat16
    wmap = _prep_weights(np.asarray(Wf), np.asarray(bf), np.asarray(Wg),
                         np.asarray(bg), np.asarray(Wh), np.asarray(bh),
                         np.asarray(gamma))
    xf = np.ascontiguousarray(np.asarray(x, np.float32).reshape(B, C, N))
    in_maps = []
    for b in range(B):
        m = dict(wmap)
        m["xb"] = xf[b].astype(bf16)
        x8 = np.empty((64, NMB, 2, MBLK), np.float32)
        for j in range(NMB):
            blk = slice(j * MBLK, (j + 1) * MBLK)
            x8[:, j, 0, :] = xf[b][0:64, blk]
            x8[:, j, 1, :] = xf[b][64:128, blk]
        m["x8"] = x8.reshape(64, 2 * N).astype(fp8)
        in_maps.append(m)
    return in_maps


def kernel(x, Wf, bf, Wg, bg, Wh, bh, gamma):
    nc = _get_nc()
    in_maps = make_in_maps(x, Wf, bf, Wg, bg, Wh, bh, gamma)
    res = run_bass_kernel_spmd(nc, in_maps, core_ids=list(range(B)))
    out = np.stack([res.results[b]["out"] for b in range(B)], axis=0)
    return out.reshape(B, C, W, H).astype(np.float32)


# revision 56
# speedup vs baseline: 1.1489x; 1.1489x over previous
"""NonLocal2D block (SAGAN-style non-local attention) on 8 Trainium2 cores.

Data-parallel over batch: core b computes batch element b entirely on-chip.

Math (per batch, N = 64*64 = 4096):
  f = Wf@x+bf [16,N], g = Wg@x+bg [16,N], h = Wh@x+bh [128,N]
  S = f^T g [N,N]; A = softmax_rows(S); att = h @ A; out = x + gamma*att

Decomposition (per core): 32 row-strips of 128 n's, in windows of 4.
Per strip:
  S_strip = f_strip^T @ g          PE, K=16 bf16 matmuls -> PSUM
  E_strip = exp(S_strip - 6)       ACT (the critical engine), PSUM->bf16
  D rowsum                         one fused DVE op: in-place identity
                                   tensor_scalar over E with accum_out
                                   (4x perf mode, ~1.1us) -- even strips
                                   first get a gpsimd halving add
                                   (E_lo+E_hi, SBUF only) so the DVE
                                   pass is 2048 wide (~0.6us); that DVE
                                   finalize is deferred one strip so
                                   DVE's in-order queue never waits on
                                   the slow Pool add.
  hT = x_strip^T @ WhT + 1 (x) bh  PE (K=128 + K=1 rank-1 bias)
  hT' = hT * (1/D) * gamma         DVE (reciprocal_approx_fast + 4x
                                   tensor_scalar)
Attended accumulates over 4-strip groups in PSUM (K-chained matmuls),
folded into an SBUF accumulator by DVE one window later; the residual
x is fused into the first fold; tail stores per 512-column block.

Engine budget (TimelineSim): ACT ~129us (86%), PE ~116us, DVE ~91us,
Pool ~72us.  ACT's exp stream is the roofline; D row-sums were moved
off the DVE tree (baseline: ~63us of adds/reduces) into fused accum
passes plus idle-gpsimd assists, which is the main win over the
original kernel.

All tensors produced/consumed blockwise are split into per-block tiles
because Tile tracks dependencies per tile, not per slice.
"""

import numpy as np
import ml_dtypes

import concourse.bass as bass
import concourse.bacc as bacc
import concourse.tile as tile
import concourse.mybir as mybir
from concourse.bass_utils import run_bass_kernel_spmd

B, C, W, H = 8, 128, 64, 64
N = W * H          # 4096
CP = 16            # f/g channels
P = 128
NSTRIP = N // P    # 32
GROUP = 4          # strips per window / attended K-chain
NWIN = NSTRIP // GROUP      # 8
MBLK = 512
NMB = N // MBLK    # 8
# exp call chunks (psS tiles are [128,1536] = 3 banks x 2 bufs):
CHUNKS = [(0, 1536), (1536, 1536), (3072, 1024)]
# rowsum: chunks 0..1 (3072 cols) via DVE add-tree, chunk 2 via ACT accum

F32 = mybir.dt.float32
BF16 = mybir.dt.bfloat16
F16 = mybir.dt.float16
EXP = mybir.ActivationFunctionType.Exp
AX = mybir.AxisListType.X
MUL = mybir.AluOpType.mult
ADD = mybir.AluOpType.add

_NC = None


def _build():
    nc = bacc.Bacc(None, target_bir_lowering=False)
    x32 = nc.dram_tensor("x32", [P, N], F32, kind="ExternalInput")
    xbf = nc.dram_tensor("xbf", [P, N], BF16, kind="ExternalInput")
    # wpack: [wf^T rep | wg^T rep | wh^T | row0: bh] packed on host
    wpack = nc.dram_tensor("wpack", [P, 4 * P], BF16, kind="ExternalInput")
    # fpack: [bf4 | bg4 | gamma (pre-broadcast)] packed on host
    fpack = nc.dram_tensor("fpack", [P, 3], F32, kind="ExternalInput")
    out = nc.dram_tensor("out", [P, N], F32, kind="ExternalOutput")

    with tile.TileContext(nc) as tc:
        with (
            tc.tile_pool(name="consts", bufs=1) as consts,
            tc.tile_pool(name="epool", bufs=2 * GROUP + 4) as epool,
            tc.tile_pool(name="hpool", bufs=2 * GROUP + 6) as hpool,
            tc.tile_pool(name="small", bufs=8) as small,
            tc.tile_pool(name="psS", bufs=2, space="PSUM") as psS,
            tc.tile_pool(name="psA", bufs=2, space="PSUM") as psA,
        ):
            # ---- interleave input DMAs across the two DGE paths so xbf
            # block 0 and the packed weights land first.
            wpack_s = consts.tile([P, 4 * P], BF16)
            fpack_s = consts.tile([P, 3], F32)
            xbf_t = [consts.tile([P, MBLK], BF16, tag=f"xbf{j}", name=f"xbf{j}")
                     for j in range(NMB)]
            nc.sync.dma_start(wpack_s[:, 0:2 * P], wpack[:, 0:2 * P])
            nc.gpsimd.dma_start(xbf_t[0][:], xbf[:, 0:MBLK])
            nc.sync.dma_start(fpack_s[:], fpack[:])
            nc.gpsimd.dma_start(wpack_s[:, 2 * P:4 * P], wpack[:, 2 * P:4 * P])
            nc.sync.dma_start(xbf_t[1][:], xbf[:, MBLK:2 * MBLK])
            for j in range(2, NMB):
                eng = nc.gpsimd if j % 2 == 0 else nc.sync
                eng.dma_start(xbf_t[j][:], xbf[:, j * MBLK:(j + 1) * MBLK])

            wft4_s = wpack_s[:, 0:P]
            wgt4_s = wpack_s[:, P:2 * P]
            wht_s = wpack_s[:, 2 * P:3 * P]
            bhr_s = wpack_s[0:1, 3 * P:4 * P]
            bf4_s = fpack_s[:, 0:1]
            bg4_s = fpack_s[:, 1:2]
            gam_s = fpack_s[:, 2:3]
            ones_s = consts.tile([1, P], BF16)
            nc.vector.memset(ones_s[:], 1.0)
            neg6_s = consts.tile([P, 1], F32)
            nc.vector.memset(neg6_s[:], -6.0)
            # dummy exp with no input deps: pulls the ACT table load to t=0
            # instead of just before the first real activation
            warm = small.tile([P, 1], F32, tag="warm")
            nc.scalar.activation(out=warm[:], in_=neg6_s[:], func=EXP)

            f4_t = [consts.tile([P, MBLK], BF16, tag=f"f4{j}", name=f"f4{j}")
                    for j in range(NMB)]
            g4_t = [consts.tile([P, wd], BF16, tag=f"g4{c}", name=f"g4{c}")
                    for c, (off, wd) in enumerate(CHUNKS)]
            att_t = [consts.tile([P, MBLK], F32, tag=f"att{j}", name=f"att{j}")
                     for j in range(NMB)]

            # ---- f/g 1x1 convs; bias added on the PSUM->SBUF copy.
            # Order matters: strip 0 needs f block 0 and the g chunks in
            # order, so emit those first; remaining f blocks trail.
            IDENT = mybir.ActivationFunctionType.Identity

            def fg_block(j, which, via_act=False):
                ps = psA.tile([P, MBLK], F32, tag="att")
                if which == "f":
                    dst, b = f4_t[j][:], bf4_s
                    nc.tensor.matmul(ps[:], wft4_s, xbf_t[j][:],
                                     start=True, stop=True)
                else:
                    c = next(i for i, (off, wd) in enumerate(CHUNKS)
                             if off <= j * MBLK < off + wd)
                    o = j * MBLK - CHUNKS[c][0]
                    dst = g4_t[c][:, o:o + MBLK]
                    b = bg4_s
                    nc.tensor.matmul(ps[:], wgt4_s, xbf_t[j][:],
                                     start=True, stop=True)
                if via_act:
                    # ACT is idle during startup; Identity shares Exp's table
                    nc.scalar.activation(out=dst, in_=ps[:], func=IDENT,
                                         bias=b, scale=1.0)
                else:
                    nc.vector.tensor_scalar_add(out=dst, in0=ps[:], scalar1=b)

            # Only what strip 0 chunk 0 needs; the rest is emitted
            # just-in-time inside the strip loop (PE executes in order, so
            # early emission would delay strip 0's S matmuls).
            fg_block(0, "f", via_act=True)
            fg_block(0, "g", via_act=False)
            fg_block(1, "g", via_act=True)
            fg_block(2, "g", via_act=False)

            # x32 only needed for the first folds; per-block tiles
            x32_t = []
            for j in range(NMB):
                t = consts.tile([P, MBLK], F32, tag=f"x32{j}", name=f"x32{j}")
                eng = nc.gpsimd if j % 2 == 0 else nc.sync
                eng.dma_start(t[:], x32[:, j * MBLK:(j + 1) * MBLK])
                x32_t.append(t)

            def att_block(j, group, first):
                """att[j] (+)= sum_k hT'_k^T @ E_k[:, blk j]; first fold also
                adds the residual x."""
                blk = slice(j * MBLK, (j + 1) * MBLK)
                pa = psA.tile([P, MBLK], F32, tag="att")
                for k, (hk, ek) in enumerate(group):
                    nc.tensor.matmul(pa[:], hk[:], ek[:, blk],
                                     start=(k == 0), stop=(k == len(group) - 1))
                if first:
                    nc.vector.tensor_add(out=att_t[j][:], in0=pa[:],
                                         in1=x32_t[j][:])
                else:
                    nc.vector.tensor_add(out=att_t[j][:], in0=att_t[j][:],
                                         in1=pa[:])

            groups = [[] for _ in range(NWIN)]
            pend_fin = []
            for w in range(NWIN):
                for i in range(GROUP):
                    s = w * GROUP + i
                    if i == 1 and w < NWIN - 1:
                        fg_block(w + 1, "f")
                    # S strip (K=16) -> exp -> E strip
                    e = epool.tile([P, N], BF16, tag="E")
                    fl = f4_t[s // 4][:, (s % 4) * P:(s % 4 + 1) * P]
                    fsl = fl[0:CP, :]
                    for cix, (coff, cwd) in enumerate(CHUNKS):
                        if s == 0 and cix >= 1:
                            for gb in range(coff // MBLK,
                                            (coff + cwd) // MBLK):
                                fg_block(gb, "g")
                        sps = psS.tile([P, 1536], F32)
                        for half in range(cwd // MBLK):
                            off = half * MBLK
                            nc.tensor.matmul(
                                sps[:, off:off + MBLK],
                                fsl,
                                g4_t[cix][0:CP, off:off + MBLK],
                                start=True, stop=True)
                        eout = e[:, coff:coff + cwd]
                        # exp(S - 6): softmax is shift-invariant and the
                        # normalization uses the same shifted sums
                        nc.scalar.activation(out=eout, in_=sps[:, 0:cwd],
                                             func=EXP, bias=neg6_s[:])
                        # attended for the previous window's group, one block
                        # after each of the first two chunks: each PE segment
                        # between S chunks then matches ACT's consumption rate
                        if w >= 1 and cix < 2:
                            att_block(2 * i + cix, groups[w - 1],
                                      first=(w == 1))
                    # hT = x_strip^T @ WhT + ones (x) bh  -> [n, c]; after
                    # the S chunks so it never delays ACT's food; copied to
                    # SBUF right away so the psA slot frees quickly
                    ph = psA.tile([P, MBLK], F32, tag="att", name="ph")
                    nc.tensor.matmul(ph[:, 0:P], xbf_t[s // 4][:, (s % 4) * P:
                                                               (s % 4 + 1) * P],
                                     wht_s, start=True, stop=False)
                    nc.tensor.matmul(ph[:, 0:P], ones_s[:], bhr_s,
                                     start=False, stop=True)
                    ht0 = hpool.tile([P, P], BF16, tag="ht0", name="ht0")
                    nc.vector.tensor_copy(out=ht0[:], in_=ph[:, 0:P])
                    # D rowsum: Pool halving add + DVE 2048-wide in-place
                    # accum pass (4x) for even strips; plain 4096-wide DVE
                    # in-place accum for odd strips.  The DVE finalize of a
                    # Pool-assisted strip is deferred one strip so DVE's
                    # in-order queue never waits on the slow Pool add.
                    if s % 2 == 0 and s < NSTRIP - 2:
                        t1 = small.tile([P, 2048], BF16, tag="t1")
                        nc.gpsimd.tensor_add(out=t1[:], in0=e[:, 0:2048],
                                             in1=e[:, 2048:4096])
                        pend_fin.append((s, t1, ht0, e))
                    else:
                        d = small.tile([P, 1], F32, tag="d")
                        nc.vector.tensor_scalar(out=e[:], in0=e[:],
                                                scalar1=1.0, scalar2=0.0,
                                                op0=MUL, op1=ADD,
                                                accum_out=d[:])
                        rd = small.tile([P, 1], F32, tag="rd")
                        nc.vector.reciprocal_approx_fast(out=rd[:], in_=d[:])
                        hts = hpool.tile([P, P], BF16, tag="hts")
                        nc.vector.tensor_scalar(out=hts[:], in0=ht0[:],
                                                scalar1=rd[:], scalar2=gam_s,
                                                op0=MUL, op1=MUL)
                        groups[w].append((hts, e))
                    while pend_fin and (pend_fin[0][0] <= s - 1
                                        or s == NSTRIP - 1):
                        s0, t1, h0, e0 = pend_fin.pop(0)
                        d = small.tile([P, 1], F32, tag="d")
                        nc.vector.tensor_scalar(out=t1[:], in0=t1[:],
                                                scalar1=1.0, scalar2=0.0,
                                                op0=MUL, op1=ADD,
                                                accum_out=d[:])
                        rd = small.tile([P, 1], F32, tag="rd")
                        nc.vector.reciprocal_approx_fast(out=rd[:], in_=d[:])
                        hts = hpool.tile([P, P], BF16, tag="hts")
                        nc.vector.tensor_scalar(out=hts[:], in0=h0[:],
                                                scalar1=rd[:], scalar2=gam_s,
                                                op0=MUL, op1=MUL)
                        groups[s0 // 4].append((hts, e0))

            # tail: attended for the last group, then store. pa tiles come
            # from both psum pools (psS is idle now) for deeper overlap.
            for j in range(NMB):
                pool = psS if j % 2 == 0 else psA
                pa = pool.tile([P, 1536 if pool is psS else MBLK], F32,
                               tag="sps" if pool is psS else "att",
                               name=f"tailpa{j}")
                g = groups[-1]
                for k, (hk, ek) in enumerate(g):
                    nc.tensor.matmul(pa[:, 0:MBLK], hk[:],
                                     ek[:, j * MBLK:(j + 1) * MBLK],
                                     start=(k == 0), stop=(k == len(g) - 1))
                nc.vector.tensor_add(out=att_t[j][:], in0=att_t[j][:],
                                     in1=pa[:, 0:MBLK])
                nc.sync.dma_start(out[:, j * MBLK:(j + 1) * MBLK], att_t[j][:])

    nc.compile()
    return nc


def _get_nc():
    global _NC
    if _NC is None:
        _NC = _build()
    return _NC


def _prep_weights(Wf, bf, Wg, bg, Wh, bh, gamma):
    bf16 = ml_dtypes.bfloat16
    wft4 = np.zeros((P, P), np.float32)
    wgt4 = np.zeros((P, P), np.float32)
    bf4 = np.zeros((P, 1), np.float32)
    bg4 = np.zeros((P, 1), np.float32)
    for i in range(4):
        wft4[:, 32 * i:32 * i + CP] = Wf.T
        wgt4[:, 32 * i:32 * i + CP] = Wg.T
        bf4[32 * i:32 * i + CP, 0] = bf
        bg4[32 * i:32 * i + CP, 0] = bg
    wpack = np.zeros((P, 4 * P), np.float32)
    wpack[:, 0:P] = wft4
    wpack[:, P:2 * P] = wgt4
    wpack[:, 2 * P:3 * P] = Wh.T
    wpack[0, 3 * P:4 * P] = bh
    fpack = np.zeros((P, 3), np.float32)
    fpack[:, 0:1] = bf4
    fpack[:, 1:2] = bg4
    fpack[:, 2] = np.float32(np.asarray(gamma).reshape(()))
    return {"wpack": wpack.astype(bf16), "fpack": fpack}


def make_in_maps(x, Wf, bf, Wg, bg, Wh, bh, gamma):
    bf16 = ml_dtypes.bfloat16
    wmap = _prep_weights(np.asarray(Wf), np.asarray(bf), np.asarray(Wg),
                         np.asarray(bg), np.asarray(Wh), np.asarray(bh),
                         np.asarray(gamma))
    xf = np.ascontiguousarray(np.asarray(x, np.float32).reshape(B, C, N))
    in_maps = []
    for b in range(B):
        m = dict(wmap)
        m["x32"] = xf[b]
        m["xbf"] = xf[b].astype(bf16)
        in_maps.append(m)
    return in_maps


def kernel(x, Wf, bf, Wg, bg, Wh, bh, gamma):
    nc = _get_nc()
    in_maps = make_in_maps(x, Wf, bf, Wg, bg, Wh, bh, gamma)
    res = run_bass_kernel_spmd(nc, in_maps, core_ids=list(range(B)))
    out = np.stack([res.results[b]["out"] for b in range(B)], axis=0)
    return out.reshape(B, C, W, H).astype(np.float32)

